# revision 41
# baseline (speedup 1.0000x reference)
"""Trainium2 Bass kernel for windowed multi-head attention.

Reference computation (per (B, N) window, P=256 tokens, C=384 channels,
H=6 heads, D=64):
    qkv  = x @ w_qkv                       # (P, 3C)
    attn = softmax((q @ k^T) * D)          # NOTE: multiplied by D=64
    out  = (attn @ v) @ w_proj + b_proj

Sharding: data-parallel over the 256 independent (B, N) windows ->
32 windows per core, weights replicated. No collectives.

Precision: q/k projection and the q@k^T logits run in fp32 (the softmax
logits span ~±1500 so absolute logit accuracy matters); the v projection,
attn@v, and output projection run in float32r (~13-bit mantissa, 4x the
TensorEngine throughput of fp32) which perturbs the output by ~1e-4.
"""

import numpy as np

import bass_rust
import concourse.bass as bass
import concourse.mybir as mybir
import concourse.tile as tile
from concourse.bass_utils import run_bass_kernel_spmd
from concourse.masks import make_identity
from concourse.vector_clock import ScopedClock

# ---------------------------------------------------------------------------
# Workaround: this walrus build only encodes one sync wait per instruction;
# TileContext's exit drain can carry one wait per active logical processor.
# Split them so each drain carries at most one.
# ---------------------------------------------------------------------------


def _split_drain_and_barrier(self, tick_clock, wait_clock):
    drain_inst = self.nc.sync.drain()
    wait_clock.add_sem_waits(
        drain_inst.ins, ScopedClock({None: tick_clock.global_clock})
    )
    si = drain_inst.ins.sync_info
    if si is not None and len(si.on_wait) > 1:
        waits = list(si.on_wait)
        updates = list(si.on_update)
        drain_inst.ins.sync_info = mybir.SyncInfo(
            on_wait=[waits[0]], on_update=updates
        )
        for w in waits[1:]:
            d2 = self.nc.sync.drain()
            d2.ins.sync_info = mybir.SyncInfo(on_wait=[w], on_update=[])

    self.nc.all_engine_barrier()
    assert self.sems is not None
    popped = self.nc._tile_sem_poison_stack.pop()
    assert popped is self._sem_poison
    self.nc.clear_and_free_semaphores(list(self.sems.allocated().values()))
    self.nc.all_engine_barrier()


tile.TileContext._drain_and_barrier = _split_drain_and_barrier

# ---------------------------------------------------------------------------

F32 = mybir.dt.float32
F32R = mybir.dt.float32r
BF16 = mybir.dt.bfloat16

B, N, P, C = 4, 64, 256, 384
H, D = 6, 64
N_CORES = 8
WINDOWS_PER_CORE = (B * N) // N_CORES       # 32
TOK = WINDOWS_PER_CORE * P                  # 8192 tokens per core
W_BLK = 4                                   # windows per block
T_BLK = W_BLK * P                           # 1024 tokens per block
N_BLOCKS = WINDOWS_PER_CORE // W_BLK        # 8
TT_BLK = T_BLK // 128                       # 8 token-tiles per block
SCALE = float(D)                            # the reference multiplies by D


def build_kernel(n_blocks=N_BLOCKS):
    nc = bass.Bass()
    x = nc.declare_dram_parameter("x", [TOK, C], F32, isOutput=False)
    w_qkv = nc.declare_dram_parameter("w_qkv", [C, 3 * C], F32, isOutput=False)
    w_proj = nc.declare_dram_parameter("w_proj", [C, C], F32, isOutput=False)
    b_proj = nc.declare_dram_parameter("b_proj", [C], F32, isOutput=False)
    out = nc.declare_dram_parameter("out", [TOK, C], F32, isOutput=True)

    with tile.TileContext(nc) as tc:
        _body(tc, x, w_qkv, w_proj, b_proj, out, n_blocks)
    # This walrus build encodes at most one sync wait per instruction; these
    # bacc passes split multi-wait instructions into event-semaphore chains.
    bass_rust.move_matmul_waits_to_ldweights(nc.m)
    bass_rust.generate_event_semaphores(nc)
    return nc


def _body(tc, x, w_qkv, w_proj, b_proj, out, n_blocks):
    nc = tc.nc
    with (
        tc.tile_pool(name="const", bufs=1) as constp,
        tc.tile_pool(name="xnat", bufs=4) as xnatp,
        tc.tile_pool(name="xT", bufs=1) as xTp,
        tc.tile_pool(name="qkT", bufs=1) as qkTp,
        tc.tile_pool(name="vsb", bufs=1) as vsbp,
        tc.tile_pool(name="outT", bufs=1) as outTp,
        tc.tile_pool(name="attsb", bufs=4) as attsbp,
        tc.tile_pool(name="small", bufs=8) as smallp,
        tc.tile_pool(name="osb", bufs=3) as osbp,
        tc.tile_pool(name="ps_mm", bufs=2, space="PSUM") as psmm,
        tc.tile_pool(name="ps_s", bufs=4, space="PSUM") as pss,
        tc.tile_pool(name="ps_aT", bufs=2, space="PSUM") as psaT,
    ):
        pso = psmm  # PV outputs ride the (idle-during-attention) mm slots
        # ---- constants ----
        ident = constp.tile([128, 128], F32)
        make_identity(nc, ident[:])
        identR = constp.tile([128, 128], F32R)
        nc.vector.tensor_copy(identR[:], ident[:])
        identB = constp.tile([128, 128], BF16)
        nc.vector.tensor_copy(identB[:], ident[:])

        wqk = constp.tile([128, 3, 2 * C], F32R)   # q,k cols of w_qkv
        wv = constp.tile([128, 3, C], F32R)        # v cols
        wp = constp.tile([128, 3, C], F32R)        # w_proj
        for cs in range(3):
            wstage = constp.tile([128, 3 * C], F32, tag="wstage")
            nc.sync.dma_start(
                out=wstage[:], in_=w_qkv[cs * 128:(cs + 1) * 128, :]
            )
            # fold the *D logit scale into the q columns once
            nc.scalar.mul(
                out=wstage[:, 0:C], in_=wstage[:, 0:C], mul=SCALE
            )
            nc.vector.tensor_copy(wqk[:, cs, :], wstage[:, 0:2 * C])
            nc.vector.tensor_copy(wv[:, cs, :], wstage[:, 2 * C:3 * C])
            wpstage = constp.tile([128, C], F32, tag="wpstage")
            nc.sync.dma_start(
                out=wpstage[:], in_=w_proj[cs * 128:(cs + 1) * 128, :]
            )
            nc.vector.tensor_copy(wp[:, cs, :], wpstage[:])

        bias = constp.tile([128, C], F32)
        nc.gpsimd.dma_start(out=bias[:, :], in_=b_proj[:].partition_broadcast(128))

        for blk in range(n_blocks):
            t0 = blk * T_BLK  # first token row of this block

            # ---- stage A: load x, transpose to [C, tokens] ----
            with nc.named_scope(f"xT{blk}"):
                xTr = xTp.tile([128, 3, T_BLK], F32R, tag="xTr")
                for tt in range(TT_BLK):
                    xin = xnatp.tile([128, C], F32, tag="xnat")
                    nc.sync.dma_start(
                        out=xin[:], in_=x[t0 + tt * 128: t0 + (tt + 1) * 128, :]
                    )
                    xps = psmm.tile([128, 3, 128], F32, tag="mm")
                    for cs in range(3):
                        nc.tensor.transpose(
                            xps[:, cs, :], xin[:, cs * 128:(cs + 1) * 128], ident[:]
                        )
                    nc.vector.tensor_copy(
                        xTr[:, :, tt * 128:(tt + 1) * 128], xps[:, :, :]
                    )

            # ---- stage B: qkT[ch, tok] = (w_qkv[:, :768]).T @ x.T ----
            with nc.named_scope(f"qk{blk}"):
                qkT = qkTp.tile([128, 6, T_BLK], F32R, tag="qkT")
                for m in range(6):
                    for nn in range(T_BLK // 512):
                        qps = psmm.tile([128, 512], F32, tag="mm")
                        for cs in range(3):
                            nc.tensor.matmul(
                                qps[:, :],
                                wqk[:, cs, m * 128:(m + 1) * 128],
                                xTr[:, cs, nn * 512:(nn + 1) * 512],
                                start=(cs == 0),
                                stop=(cs == 2),
                            )
                        nc.scalar.copy(
                            out=qkT[:, m, nn * 512:(nn + 1) * 512], in_=qps[:, :]
                        )

            # ---- stage C: v[tok, vch] = x @ w_v ----
            with nc.named_scope(f"v{blk}"):
                vsb = vsbp.tile([128, TT_BLK, C], BF16, tag="vsb")
                for tt in range(TT_BLK):
                    vps = psmm.tile([128, C], F32, tag="mm")
                    for cs in range(3):
                        nc.tensor.matmul(
                            vps[:, :],
                            xTr[:, cs, tt * 128:(tt + 1) * 128],
                            wv[:, cs, :],
                            start=(cs == 0),
                            stop=(cs == 2),
                        )
                    nc.vector.tensor_copy(vsb[:, tt, :], vps[:, :])

            # ---- stage D: attention, software-pipelined one head-pair
            # ahead: pair n+1's score matmuls are issued before pair n's
            # softmax/transpose/PV so the PE fills the softmax latency. ----
            outT = outTp.tile([128, 3, T_BLK], F32R, tag="outT")

            def d_scores(wi, hp):
                wc = wi * P
                mq, mk = hp, 3 + hp
                with nc.named_scope(f"score{blk}"):
                    spair = [
                        pss.tile([128, 2, P], F32, tag="s", name=f"s{hh}")
                        for hh in range(2)
                    ]
                    for qt in range(2):
                        for hh in range(2):
                            po = 64 * hh
                            nc.tensor.matmul(
                                spair[hh][:, qt, :],
                                qkT[po:po + 64, mq,
                                    wc + qt * 128: wc + (qt + 1) * 128],
                                qkT[po:po + 64, mk, wc: wc + P],
                                start=True,
                                stop=True,
                                tile_position=(po, 0),
                            )
                return spair

            def d_rest(wi, hp, spair):
                wc = wi * P
                with nc.named_scope(f"smax{blk}"):
                    epair = []
                    for hh in range(2):
                        sps = spair[hh]
                        expS = attsbp.tile([128, 2, P], F32, tag="expS")
                        expR = attsbp.tile([128, 2, P], BF16, tag="expR")
                        epair.append(expR)
                        rs = smallp.tile([128, 4], F32, tag="rs")
                        nc.vector.reduce_max(
                            rs[:, 0:2], sps[:, :, :],
                            axis=mybir.AxisListType.X, negate=True,
                        )
                        for qt in range(2):
                            nc.scalar.activation(
                                expS[:, qt, :], sps[:, qt, :],
                                mybir.ActivationFunctionType.Exp,
                                bias=rs[:, qt:qt + 1],
                                accum_out=rs[:, 2 + qt:3 + qt],
                            )
                        nc.vector.reciprocal(rs[:, 2:4], rs[:, 2:4])
                        for qt in range(2):
                            nc.vector.tensor_scalar_mul(
                                expR[:, qt, :], expS[:, qt, :],
                                rs[:, 2 + qt:3 + qt]
                            )
                with nc.named_scope(f"atr{blk}"):
                    apair = []
                    for hh in range(2):
                        expR = epair[hh]
                        aTps = psaT.tile([128, 2, P], BF16, tag="aT")
                        for qt in range(2):
                            for kb in range(2):
                                nc.tensor.transpose(
                                    aTps[:, kb, qt * 128:(qt + 1) * 128],
                                    expR[:, qt, kb * 128:(kb + 1) * 128],
                                    identB[:],
                                )
                        aT = attsbp.tile([128, 2, P], BF16, tag="aTsb")
                        apair.append(aT)
                        nc.vector.tensor_copy(aT[:, 0, :], aTps[:, 0, :])
                        nc.scalar.copy(out=aT[:, 1, :], in_=aTps[:, 1, :])
                with nc.named_scope(f"pv{blk}"):
                    # full-M PV: lhsT spans the head PAIR's v channels so
                    # tile_position stays (0,0) (f32r rejects col offsets);
                    # each head's matmul yields its 64 valid output rows,
                    # the other 64 are discarded.
                    opair = []
                    for hh in range(2):
                        ops = pso.tile([128, P], F32, tag="mm")
                        opair.append(ops)
                        for kb in range(2):
                            nc.tensor.matmul(
                                ops[:, :],
                                vsb[:, wi * 2 + kb, 128 * hp: 128 * (hp + 1)],
                                apair[hh][:, kb, :],
                                start=(kb == 0),
                                stop=(kb == 1),
                            )
                    nc.vector.tensor_copy(
                        outT[0:64, hp, wc: wc + P], opair[0][0:64, :]
                    )
                    nc.vector.tensor_copy(
                        outT[64:128, hp, wc: wc + P], opair[1][64:128, :]
                    )

            pending = None
            for wi in range(W_BLK):
                for hp in range(3):
                    sp = d_scores(wi, hp)
                    if pending is not None:
                        d_rest(*pending)
                    pending = (wi, hp, sp)
            d_rest(*pending)

            # ---- stage E: proj + bias ----
            with nc.named_scope(f"proj{blk}"):
                for tt in range(TT_BLK):
                    pps = psmm.tile([128, C], F32, tag="mm")
                    for cs in range(3):
                        nc.tensor.matmul(
                            pps[:, :],
                            outT[:, cs, tt * 128:(tt + 1) * 128],
                            wp[:, cs, :],
                            start=(cs == 0),
                            stop=(cs == 2),
                        )
                    osb = osbp.tile([128, C], F32, tag="osb")
                    nc.vector.tensor_add(osb[:, :], pps[:, :], bias[:, :])
                    nc.sync.dma_start(
                        out=out[t0 + tt * 128: t0 + (tt + 1) * 128, :], in_=osb[:]
                    )


_NC_CACHE = {}


def _get_nc(n_blocks=N_BLOCKS):
    if n_blocks not in _NC_CACHE:
        _NC_CACHE[n_blocks] = build_kernel(n_blocks)
    return _NC_CACHE[n_blocks]


def kernel(x, w_qkv, w_proj, b_proj, **_run_kw):
    x = np.ascontiguousarray(np.asarray(x, dtype=np.float32))
    w_qkv = np.ascontiguousarray(np.asarray(w_qkv, dtype=np.float32))
    w_proj = np.ascontiguousarray(np.asarray(w_proj, dtype=np.float32))
    b_proj = np.ascontiguousarray(np.asarray(b_proj, dtype=np.float32))

    xf = x.reshape(B * N * P, C)
    in_maps = [
        {
            "x": xf[c * TOK:(c + 1) * TOK],
            "w_qkv": w_qkv,
            "w_proj": w_proj,
            "b_proj": b_proj,
        }
        for c in range(N_CORES)
    ]
    nc = _get_nc()
    res = run_bass_kernel_spmd(nc, in_maps, core_ids=list(range(N_CORES)), **_run_kw)
    outf = np.concatenate([res.results[c]["out"] for c in range(N_CORES)], axis=0)
    result = outf.reshape(B, N, P, C)
    if _run_kw:
        return result, res
    return result


# revision 43
# speedup vs baseline: 1.0337x; 1.0337x over previous
"""Trainium2 Bass kernel for windowed multi-head attention.

Reference computation (per (B, N) window, P=256 tokens, C=384 channels,
H=6 heads, D=64):
    qkv  = x @ w_qkv                       # (P, 3C)
    attn = softmax((q @ k^T) * D)          # NOTE: multiplied by D=64
    out  = (attn @ v) @ w_proj + b_proj

Sharding: data-parallel over the 256 independent (B, N) windows ->
32 windows per core, weights replicated. No collectives.

Precision: q/k projection and the q@k^T logits run in fp32 (the softmax
logits span ~±1500 so absolute logit accuracy matters); the v projection,
attn@v, and output projection run in float32r (~13-bit mantissa, 4x the
TensorEngine throughput of fp32) which perturbs the output by ~1e-4.
"""

import numpy as np

import bass_rust
import concourse.bass as bass
import concourse.mybir as mybir
import concourse.tile as tile
from concourse.bass_utils import run_bass_kernel_spmd
from concourse.masks import make_identity
from concourse.vector_clock import ScopedClock

# ---------------------------------------------------------------------------
# Workaround: this walrus build only encodes one sync wait per instruction;
# TileContext's exit drain can carry one wait per active logical processor.
# Split them so each drain carries at most one.
# ---------------------------------------------------------------------------


def _split_drain_and_barrier(self, tick_clock, wait_clock):
    drain_inst = self.nc.sync.drain()
    wait_clock.add_sem_waits(
        drain_inst.ins, ScopedClock({None: tick_clock.global_clock})
    )
    si = drain_inst.ins.sync_info
    if si is not None and len(si.on_wait) > 1:
        waits = list(si.on_wait)
        updates = list(si.on_update)
        drain_inst.ins.sync_info = mybir.SyncInfo(
            on_wait=[waits[0]], on_update=updates
        )
        for w in waits[1:]:
            d2 = self.nc.sync.drain()
            d2.ins.sync_info = mybir.SyncInfo(on_wait=[w], on_update=[])

    self.nc.all_engine_barrier()
    assert self.sems is not None
    popped = self.nc._tile_sem_poison_stack.pop()
    assert popped is self._sem_poison
    self.nc.clear_and_free_semaphores(list(self.sems.allocated().values()))
    self.nc.all_engine_barrier()


tile.TileContext._drain_and_barrier = _split_drain_and_barrier

# ---------------------------------------------------------------------------

F32 = mybir.dt.float32
F32R = mybir.dt.float32r
BF16 = mybir.dt.bfloat16

B, N, P, C = 4, 64, 256, 384
H, D = 6, 64
N_CORES = 8
WINDOWS_PER_CORE = (B * N) // N_CORES       # 32
TOK = WINDOWS_PER_CORE * P                  # 8192 tokens per core
W_BLK = 4                                   # windows per block
T_BLK = W_BLK * P                           # 1024 tokens per block
N_BLOCKS = WINDOWS_PER_CORE // W_BLK        # 8
TT_BLK = T_BLK // 128                       # 8 token-tiles per block
SCALE = float(D)                            # the reference multiplies by D


def build_kernel(n_blocks=N_BLOCKS):
    nc = bass.Bass()
    x = nc.declare_dram_parameter("x", [TOK, C], F32, isOutput=False)
    w_qkv = nc.declare_dram_parameter("w_qkv", [C, 3 * C], F32, isOutput=False)
    w_proj = nc.declare_dram_parameter("w_proj", [C, C], F32, isOutput=False)
    b_proj = nc.declare_dram_parameter("b_proj", [C], F32, isOutput=False)
    out = nc.declare_dram_parameter("out", [TOK, C], F32, isOutput=True)

    with tile.TileContext(nc) as tc:
        _body(tc, x, w_qkv, w_proj, b_proj, out, n_blocks)
    # This walrus build encodes at most one sync wait per instruction; these
    # bacc passes split multi-wait instructions into event-semaphore chains.
    bass_rust.move_matmul_waits_to_ldweights(nc.m)
    bass_rust.generate_event_semaphores(nc)
    return nc


def _body(tc, x, w_qkv, w_proj, b_proj, out, n_blocks):
    nc = tc.nc
    with (
        tc.tile_pool(name="const", bufs=1) as constp,
        tc.tile_pool(name="xnat", bufs=4) as xnatp,
        tc.tile_pool(name="xT", bufs=1) as xTp,
        tc.tile_pool(name="qkT", bufs=1) as qkTp,
        tc.tile_pool(name="vsb", bufs=1) as vsbp,
        tc.tile_pool(name="outT", bufs=1) as outTp,
        tc.tile_pool(name="attsb", bufs=4) as attsbp,
        tc.tile_pool(name="small", bufs=8) as smallp,
        tc.tile_pool(name="osb", bufs=3) as osbp,
        tc.tile_pool(name="ps_mm", bufs=2, space="PSUM") as psmm,
        tc.tile_pool(name="ps_s", bufs=4, space="PSUM") as pss,
        tc.tile_pool(name="ps_aT", bufs=2, space="PSUM") as psaT,
    ):
        pso = psmm  # PV outputs ride the (idle-during-attention) mm slots
        # ---- constants ----
        ident = constp.tile([128, 128], F32)
        make_identity(nc, ident[:])
        identR = constp.tile([128, 128], F32R)
        nc.vector.tensor_copy(identR[:], ident[:])
        identB = constp.tile([128, 128], BF16)
        nc.vector.tensor_copy(identB[:], ident[:])

        wqk = constp.tile([128, 3, 2 * C], F32R)   # q,k cols of w_qkv
        wv = constp.tile([128, 3, C], F32R)        # v cols
        wp = constp.tile([128, 3, C], F32R)        # w_proj
        for cs in range(3):
            wstage = constp.tile([128, 3 * C], F32, tag="wstage")
            nc.sync.dma_start(
                out=wstage[:], in_=w_qkv[cs * 128:(cs + 1) * 128, :]
            )
            # fold the *D logit scale into the q columns once
            nc.scalar.mul(
                out=wstage[:, 0:C], in_=wstage[:, 0:C], mul=SCALE
            )
            nc.vector.tensor_copy(wqk[:, cs, :], wstage[:, 0:2 * C])
            nc.vector.tensor_copy(wv[:, cs, :], wstage[:, 2 * C:3 * C])
            wpstage = constp.tile([128, C], F32, tag="wpstage")
            nc.sync.dma_start(
                out=wpstage[:], in_=w_proj[cs * 128:(cs + 1) * 128, :]
            )
            nc.vector.tensor_copy(wp[:, cs, :], wpstage[:])

        bias = constp.tile([128, C], F32)
        nc.gpsimd.dma_start(out=bias[:, :], in_=b_proj[:].partition_broadcast(128))

        for blk in range(n_blocks):
            t0 = blk * T_BLK  # first token row of this block

            # ---- stage A: load x, transpose to [C, tokens] ----
            with nc.named_scope(f"xT{blk}"):
                xTr = xTp.tile([128, 3, T_BLK], F32R, tag="xTr")
                for tt in range(TT_BLK):
                    xin = xnatp.tile([128, C], F32, tag="xnat")
                    nc.sync.dma_start(
                        out=xin[:], in_=x[t0 + tt * 128: t0 + (tt + 1) * 128, :]
                    )
                    xps = psmm.tile([128, 3, 128], F32, tag="mm")
                    for cs in range(3):
                        nc.tensor.transpose(
                            xps[:, cs, :], xin[:, cs * 128:(cs + 1) * 128], ident[:]
                        )
                    nc.scalar.copy(
                        out=xTr[:, :, tt * 128:(tt + 1) * 128], in_=xps[:, :, :]
                    )

            # ---- stage B: qkT[ch, tok] = (w_qkv[:, :768]).T @ x.T ----
            with nc.named_scope(f"qk{blk}"):
                qkT = qkTp.tile([128, 6, T_BLK], F32R, tag="qkT")
                for m in range(6):
                    for nn in range(T_BLK // 512):
                        qps = psmm.tile([128, 512], F32, tag="mm")
                        for cs in range(3):
                            nc.tensor.matmul(
                                qps[:, :],
                                wqk[:, cs, m * 128:(m + 1) * 128],
                                xTr[:, cs, nn * 512:(nn + 1) * 512],
                                start=(cs == 0),
                                stop=(cs == 2),
                            )
                        nc.scalar.copy(
                            out=qkT[:, m, nn * 512:(nn + 1) * 512], in_=qps[:, :]
                        )

            # ---- stage C: v[tok, vch] = x @ w_v ----
            with nc.named_scope(f"v{blk}"):
                vsb = vsbp.tile([128, TT_BLK, C], BF16, tag="vsb")
                for tt in range(TT_BLK):
                    vps = psmm.tile([128, C], F32, tag="mm")
                    for cs in range(3):
                        nc.tensor.matmul(
                            vps[:, :],
                            xTr[:, cs, tt * 128:(tt + 1) * 128],
                            wv[:, cs, :],
                            start=(cs == 0),
                            stop=(cs == 2),
                        )
                    nc.vector.tensor_copy(vsb[:, tt, :], vps[:, :])

            # ---- stage D: attention, software-pipelined one head-pair
            # ahead: pair n+1's score matmuls are issued before pair n's
            # softmax/transpose/PV so the PE fills the softmax latency. ----
            outT = outTp.tile([128, 3, T_BLK], F32R, tag="outT")

            def d_scores(wi, hp):
                wc = wi * P
                mq, mk = hp, 3 + hp
                with nc.named_scope(f"score{blk}"):
                    spair = [
                        pss.tile([128, 2, P], F32, tag="s", name=f"s{hh}")
                        for hh in range(2)
                    ]
                    for qt in range(2):
                        for hh in range(2):
                            po = 64 * hh
                            nc.tensor.matmul(
                                spair[hh][:, qt, :],
                                qkT[po:po + 64, mq,
                                    wc + qt * 128: wc + (qt + 1) * 128],
                                qkT[po:po + 64, mk, wc: wc + P],
                                start=True,
                                stop=True,
                                tile_position=(po, 0),
                            )
                return spair

            def d_rest(wi, hp, spair):
                wc = wi * P
                with nc.named_scope(f"smax{blk}"):
                    epair = []
                    for hh in range(2):
                        sps = spair[hh]
                        expS = attsbp.tile([128, 2, P], BF16, tag="expS")
                        expR = attsbp.tile([128, 2, P], BF16, tag="expR")
                        epair.append(expR)
                        rs = smallp.tile([128, 4], F32, tag="rs")
                        nc.vector.reduce_max(
                            rs[:, 0:2], sps[:, :, :],
                            axis=mybir.AxisListType.X, negate=True,
                        )
                        for qt in range(2):
                            nc.scalar.activation(
                                expS[:, qt, :], sps[:, qt, :],
                                mybir.ActivationFunctionType.Exp,
                                bias=rs[:, qt:qt + 1],
                                accum_out=rs[:, 2 + qt:3 + qt],
                            )
                        nc.vector.reciprocal(rs[:, 2:4], rs[:, 2:4])
                        for qt in range(2):
                            nc.vector.tensor_scalar_mul(
                                expR[:, qt, :], expS[:, qt, :],
                                rs[:, 2 + qt:3 + qt]
                            )
                with nc.named_scope(f"atr{blk}"):
                    apair = []
                    for hh in range(2):
                        expR = epair[hh]
                        aTps = psaT.tile([128, 2, P], BF16, tag="aT")
                        for qt in range(2):
                            for kb in range(2):
                                nc.tensor.transpose(
                                    aTps[:, kb, qt * 128:(qt + 1) * 128],
                                    expR[:, qt, kb * 128:(kb + 1) * 128],
                                    identB[:],
                                )
                        aT = attsbp.tile([128, 2, P], BF16, tag="aTsb")
                        apair.append(aT)
                        nc.vector.tensor_copy(aT[:, 0, :], aTps[:, 0, :])
                        nc.scalar.copy(out=aT[:, 1, :], in_=aTps[:, 1, :])
                with nc.named_scope(f"pv{blk}"):
                    # full-M PV: lhsT spans the head PAIR's v channels so
                    # tile_position stays (0,0) (f32r rejects col offsets);
                    # each head's matmul yields its 64 valid output rows,
                    # the other 64 are discarded.
                    opair = []
                    for hh in range(2):
                        ops = pso.tile([128, P], F32, tag="mm")
                        opair.append(ops)
                        for kb in range(2):
                            nc.tensor.matmul(
                                ops[:, :],
                                vsb[:, wi * 2 + kb, 128 * hp: 128 * (hp + 1)],
                                apair[hh][:, kb, :],
                                start=(kb == 0),
                                stop=(kb == 1),
                            )
                    nc.vector.tensor_copy(
                        outT[0:64, hp, wc: wc + P], opair[0][0:64, :]
                    )
                    nc.vector.tensor_copy(
                        outT[64:128, hp, wc: wc + P], opair[1][64:128, :]
                    )

            pending = None
            for wi in range(W_BLK):
                for hp in range(3):
                    sp = d_scores(wi, hp)
                    if pending is not None:
                        d_rest(*pending)
                    pending = (wi, hp, sp)
            d_rest(*pending)

            # ---- stage E: proj + bias ----
            with nc.named_scope(f"proj{blk}"):
                for tt in range(TT_BLK):
                    pps = psmm.tile([128, C], F32, tag="mm")
                    for cs in range(3):
                        nc.tensor.matmul(
                            pps[:, :],
                            outT[:, cs, tt * 128:(tt + 1) * 128],
                            wp[:, cs, :],
                            start=(cs == 0),
                            stop=(cs == 2),
                        )
                    osb = osbp.tile([128, C], F32, tag="osb")
                    nc.vector.tensor_add(osb[:, :], pps[:, :], bias[:, :])
                    nc.sync.dma_start(
                        out=out[t0 + tt * 128: t0 + (tt + 1) * 128, :], in_=osb[:]
                    )


_NC_CACHE = {}


def _get_nc(n_blocks=N_BLOCKS):
    if n_blocks not in _NC_CACHE:
        _NC_CACHE[n_blocks] = build_kernel(n_blocks)
    return _NC_CACHE[n_blocks]


def kernel(x, w_qkv, w_proj, b_proj, **_run_kw):
    x = np.ascontiguousarray(np.asarray(x, dtype=np.float32))
    w_qkv = np.ascontiguousarray(np.asarray(w_qkv, dtype=np.float32))
    w_proj = np.ascontiguousarray(np.asarray(w_proj, dtype=np.float32))
    b_proj = np.ascontiguousarray(np.asarray(b_proj, dtype=np.float32))

    xf = x.reshape(B * N * P, C)
    in_maps = [
        {
            "x": xf[c * TOK:(c + 1) * TOK],
            "w_qkv": w_qkv,
            "w_proj": w_proj,
            "b_proj": b_proj,
        }
        for c in range(N_CORES)
    ]
    nc = _get_nc()
    res = run_bass_kernel_spmd(nc, in_maps, core_ids=list(range(N_CORES)), **_run_kw)
    outf = np.concatenate([res.results[c]["out"] for c in range(N_CORES)], axis=0)
    result = outf.reshape(B, N, P, C)
    if _run_kw:
        return result, res
    return result


# revision 45
# speedup vs baseline: 1.1227x; 1.0861x over previous
"""Trainium2 Bass kernel for windowed multi-head attention.

Reference computation (per (B, N) window, P=256 tokens, C=384 channels,
H=6 heads, D=64):
    qkv  = x @ w_qkv                       # (P, 3C)
    attn = softmax((q @ k^T) * D)          # NOTE: multiplied by D=64
    out  = (attn @ v) @ w_proj + b_proj

Sharding: data-parallel over the 256 independent (B, N) windows ->
32 windows per core, weights replicated. No collectives.

Precision: q/k projection and the q@k^T logits run in fp32 (the softmax
logits span ~±1500 so absolute logit accuracy matters); the v projection,
attn@v, and output projection run in float32r (~13-bit mantissa, 4x the
TensorEngine throughput of fp32) which perturbs the output by ~1e-4.
"""

import numpy as np

import bass_rust
import concourse.bass as bass
import concourse.mybir as mybir
import concourse.tile as tile
from concourse.bass_utils import run_bass_kernel_spmd
from concourse.masks import make_identity
from concourse.vector_clock import ScopedClock

# ---------------------------------------------------------------------------
# Workaround: this walrus build only encodes one sync wait per instruction;
# TileContext's exit drain can carry one wait per active logical processor.
# Split them so each drain carries at most one.
# ---------------------------------------------------------------------------


def _split_drain_and_barrier(self, tick_clock, wait_clock):
    drain_inst = self.nc.sync.drain()
    wait_clock.add_sem_waits(
        drain_inst.ins, ScopedClock({None: tick_clock.global_clock})
    )
    si = drain_inst.ins.sync_info
    if si is not None and len(si.on_wait) > 1:
        waits = list(si.on_wait)
        updates = list(si.on_update)
        drain_inst.ins.sync_info = mybir.SyncInfo(
            on_wait=[waits[0]], on_update=updates
        )
        for w in waits[1:]:
            d2 = self.nc.sync.drain()
            d2.ins.sync_info = mybir.SyncInfo(on_wait=[w], on_update=[])

    self.nc.all_engine_barrier()
    assert self.sems is not None
    popped = self.nc._tile_sem_poison_stack.pop()
    assert popped is self._sem_poison
    self.nc.clear_and_free_semaphores(list(self.sems.allocated().values()))
    self.nc.all_engine_barrier()


tile.TileContext._drain_and_barrier = _split_drain_and_barrier

# ---------------------------------------------------------------------------

F32 = mybir.dt.float32
F32R = mybir.dt.float32r
BF16 = mybir.dt.bfloat16

B, N, P, C = 4, 64, 256, 384
H, D = 6, 64
N_CORES = 8
WINDOWS_PER_CORE = (B * N) // N_CORES       # 32
TOK = WINDOWS_PER_CORE * P                  # 8192 tokens per core
W_BLK = 4                                   # windows per block
T_BLK = W_BLK * P                           # 1024 tokens per block
N_BLOCKS = WINDOWS_PER_CORE // W_BLK        # 8
TT_BLK = T_BLK // 128                       # 8 token-tiles per block
SCALE = float(D)                            # the reference multiplies by D


def build_kernel(n_blocks=N_BLOCKS):
    nc = bass.Bass()
    x = nc.declare_dram_parameter("x", [TOK, C], F32, isOutput=False)
    w_qkv = nc.declare_dram_parameter("w_qkv", [C, 3 * C], F32, isOutput=False)
    w_proj = nc.declare_dram_parameter("w_proj", [C, C], F32, isOutput=False)
    b_proj = nc.declare_dram_parameter("b_proj", [C], F32, isOutput=False)
    out = nc.declare_dram_parameter("out", [TOK, C], F32, isOutput=True)

    with tile.TileContext(nc) as tc:
        _body(tc, x, w_qkv, w_proj, b_proj, out, n_blocks)
    # This walrus build encodes at most one sync wait per instruction; these
    # bacc passes split multi-wait instructions into event-semaphore chains.
    bass_rust.move_matmul_waits_to_ldweights(nc.m)
    bass_rust.generate_event_semaphores(nc)
    return nc


def _body(tc, x, w_qkv, w_proj, b_proj, out, n_blocks):
    nc = tc.nc
    with (
        tc.tile_pool(name="const", bufs=1) as constp,
        tc.tile_pool(name="xnat", bufs=4) as xnatp,
        tc.tile_pool(name="xT", bufs=1) as xTp,
        tc.tile_pool(name="qkT", bufs=1) as qkTp,
        tc.tile_pool(name="vsb", bufs=1) as vsbp,
        tc.tile_pool(name="outT", bufs=1) as outTp,
        tc.tile_pool(name="attsb", bufs=4) as attsbp,
        tc.tile_pool(name="small", bufs=8) as smallp,
        tc.tile_pool(name="osb", bufs=3) as osbp,
        tc.tile_pool(name="ps_mm", bufs=2, space="PSUM") as psmm,
        tc.tile_pool(name="ps_s", bufs=4, space="PSUM") as pss,
        tc.tile_pool(name="ps_aT", bufs=2, space="PSUM") as psaT,
    ):
        pso = psmm  # PV outputs ride the (idle-during-attention) mm slots
        # ---- constants ----
        ident = constp.tile([128, 128], F32)
        make_identity(nc, ident[:])
        identR = constp.tile([128, 128], F32R)
        nc.vector.tensor_copy(identR[:], ident[:])
        identB = constp.tile([128, 128], BF16)
        nc.vector.tensor_copy(identB[:], ident[:])

        wqk = constp.tile([128, 3, 2 * C], F32R)   # q,k cols of w_qkv
        wv = constp.tile([128, 3, C], F32R)        # v cols
        wp = constp.tile([128, 3, C], F32R)        # w_proj
        for cs in range(3):
            wstage = constp.tile([128, 3 * C], F32, tag="wstage")
            nc.sync.dma_start(
                out=wstage[:], in_=w_qkv[cs * 128:(cs + 1) * 128, :]
            )
            # fold the *D logit scale into the q columns once
            nc.scalar.mul(
                out=wstage[:, 0:C], in_=wstage[:, 0:C], mul=SCALE
            )
            nc.vector.tensor_copy(wqk[:, cs, :], wstage[:, 0:2 * C])
            nc.vector.tensor_copy(wv[:, cs, :], wstage[:, 2 * C:3 * C])
            wpstage = constp.tile([128, C], F32, tag="wpstage")
            nc.sync.dma_start(
                out=wpstage[:], in_=w_proj[cs * 128:(cs + 1) * 128, :]
            )
            nc.vector.tensor_copy(wp[:, cs, :], wpstage[:])

        bias = constp.tile([128, C], F32)
        nc.gpsimd.dma_start(out=bias[:, :], in_=b_proj[:].partition_broadcast(128))

        for blk in range(n_blocks):
            t0 = blk * T_BLK  # first token row of this block

            # ---- stage A: load x, transpose to [C, tokens] ----
            with nc.named_scope(f"xT{blk}"):
                xTr = xTp.tile([128, 3, T_BLK], F32R, tag="xTr")
                for tt in range(TT_BLK):
                    xin = xnatp.tile([128, C], F32, tag="xnat")
                    nc.sync.dma_start(
                        out=xin[:], in_=x[t0 + tt * 128: t0 + (tt + 1) * 128, :]
                    )
                    xps = psmm.tile([128, 3, 128], F32, tag="mm")
                    for cs in range(3):
                        nc.tensor.transpose(
                            xps[:, cs, :], xin[:, cs * 128:(cs + 1) * 128], ident[:]
                        )
                    nc.scalar.copy(
                        out=xTr[:, :, tt * 128:(tt + 1) * 128], in_=xps[:, :, :]
                    )

            # ---- stage B: qkT[ch, tok] = (w_qkv[:, :768]).T @ x.T ----
            with nc.named_scope(f"qk{blk}"):
                qkT = qkTp.tile([128, 6, T_BLK], F32R, tag="qkT")
                for m in range(6):
                    for nn in range(T_BLK // 512):
                        qps = psmm.tile([128, 512], F32, tag="mm")
                        for cs in range(3):
                            nc.tensor.matmul(
                                qps[:, :],
                                wqk[:, cs, m * 128:(m + 1) * 128],
                                xTr[:, cs, nn * 512:(nn + 1) * 512],
                                start=(cs == 0),
                                stop=(cs == 2),
                            )
                        nc.scalar.copy(
                            out=qkT[:, m, nn * 512:(nn + 1) * 512], in_=qps[:, :]
                        )

            # ---- stage C: v[tok, vch] = x @ w_v ----
            with nc.named_scope(f"v{blk}"):
                vsb = vsbp.tile([128, TT_BLK, C], BF16, tag="vsb")
                for tt in range(TT_BLK):
                    vps = psmm.tile([128, C], F32, tag="mm")
                    for cs in range(3):
                        nc.tensor.matmul(
                            vps[:, :],
                            xTr[:, cs, tt * 128:(tt + 1) * 128],
                            wv[:, cs, :],
                            start=(cs == 0),
                            stop=(cs == 2),
                        )
                    nc.vector.tensor_copy(vsb[:, tt, :], vps[:, :])

            # ---- stage D: attention, software-pipelined one head-pair
            # ahead: pair n+1's score matmuls are issued before pair n's
            # softmax/transpose/PV so the PE fills the softmax latency. ----
            outT = outTp.tile([128, 3, T_BLK], F32R, tag="outT")

            def d_scores(wi, hp):
                wc = wi * P
                mq, mk = hp, 3 + hp
                with nc.named_scope(f"score{blk}"):
                    spair = [
                        pss.tile([128, 2, P], F32, tag="s", name=f"s{hh}")
                        for hh in range(2)
                    ]
                    for qt in range(2):
                        for hh in range(2):
                            po = 64 * hh
                            nc.tensor.matmul(
                                spair[hh][:, qt, :],
                                qkT[po:po + 64, mq,
                                    wc + qt * 128: wc + (qt + 1) * 128],
                                qkT[po:po + 64, mk, wc: wc + P],
                                start=True,
                                stop=True,
                                tile_position=(po, 0),
                            )
                return spair

            def d_smax(wi, hp, spair):
                with nc.named_scope(f"smax{blk}"):
                    epair = []
                    for hh in range(2):
                        sps = spair[hh]
                        expS = attsbp.tile([128, 2, P], BF16, tag="expS")
                        expR = attsbp.tile([128, 2, P], BF16, tag="expR")
                        epair.append(expR)
                        rs = smallp.tile([128, 4], F32, tag="rs")
                        nc.vector.reduce_max(
                            rs[:, 0:2], sps[:, :, :],
                            axis=mybir.AxisListType.X, negate=True,
                        )
                        for qt in range(2):
                            nc.scalar.activation(
                                expS[:, qt, :], sps[:, qt, :],
                                mybir.ActivationFunctionType.Exp,
                                bias=rs[:, qt:qt + 1],
                                accum_out=rs[:, 2 + qt:3 + qt],
                            )
                        nc.vector.reciprocal(rs[:, 2:4], rs[:, 2:4])
                        for qt in range(2):
                            nc.vector.tensor_scalar_mul(
                                expR[:, qt, :], expS[:, qt, :],
                                rs[:, 2 + qt:3 + qt]
                            )
                return epair

            def d_atrpv(wi, hp, epair):
                wc = wi * P
                with nc.named_scope(f"atr{blk}"):
                    apair = []
                    for hh in range(2):
                        expR = epair[hh]
                        aTps = psaT.tile([128, 2, P], BF16, tag="aT")
                        for qt in range(2):
                            for kb in range(2):
                                nc.tensor.transpose(
                                    aTps[:, kb, qt * 128:(qt + 1) * 128],
                                    expR[:, qt, kb * 128:(kb + 1) * 128],
                                    identB[:],
                                )
                        aT = attsbp.tile([128, 2, P], BF16, tag="aTsb")
                        apair.append(aT)
                        nc.vector.tensor_copy(aT[:, 0, :], aTps[:, 0, :])
                        nc.scalar.copy(out=aT[:, 1, :], in_=aTps[:, 1, :])
                with nc.named_scope(f"pv{blk}"):
                    # full-M PV: lhsT spans the head PAIR's v channels so
                    # tile_position stays (0,0) (f32r rejects col offsets);
                    # each head's matmul yields its 64 valid output rows,
                    # the other 64 are discarded.
                    opair = []
                    for hh in range(2):
                        ops = pso.tile([128, P], F32, tag="mm")
                        opair.append(ops)
                        for kb in range(2):
                            nc.tensor.matmul(
                                ops[:, :],
                                vsb[:, wi * 2 + kb, 128 * hp: 128 * (hp + 1)],
                                apair[hh][:, kb, :],
                                start=(kb == 0),
                                stop=(kb == 1),
                            )
                    nc.vector.tensor_copy(
                        outT[0:64, hp, wc: wc + P], opair[0][0:64, :]
                    )
                    nc.vector.tensor_copy(
                        outT[64:128, hp, wc: wc + P], opair[1][64:128, :]
                    )

            # two-stage pipeline: scores(k) | softmax(k-1) | transpose+PV(k-2)
            p_smax = None
            p_pe = None
            for wi in range(W_BLK):
                for hp in range(3):
                    sp = d_scores(wi, hp)
                    if p_smax is not None:
                        ep = d_smax(*p_smax)
                        if p_pe is not None:
                            d_atrpv(*p_pe)
                        p_pe = (p_smax[0], p_smax[1], ep)
                    p_smax = (wi, hp, sp)
            ep = d_smax(*p_smax)
            d_atrpv(*p_pe)
            d_atrpv(p_smax[0], p_smax[1], ep)

            # ---- stage E: proj + bias ----
            with nc.named_scope(f"proj{blk}"):
                for tt in range(TT_BLK):
                    pps = psmm.tile([128, C], F32, tag="mm")
                    for cs in range(3):
                        nc.tensor.matmul(
                            pps[:, :],
                            outT[:, cs, tt * 128:(tt + 1) * 128],
                            wp[:, cs, :],
                            start=(cs == 0),
                            stop=(cs == 2),
                        )
                    osb = osbp.tile([128, C], F32, tag="osb")
                    nc.vector.tensor_add(osb[:, :], pps[:, :], bias[:, :])
                    nc.sync.dma_start(
                        out=out[t0 + tt * 128: t0 + (tt + 1) * 128, :], in_=osb[:]
                    )


_NC_CACHE = {}


def _get_nc(n_blocks=N_BLOCKS):
    if n_blocks not in _NC_CACHE:
        _NC_CACHE[n_blocks] = build_kernel(n_blocks)
    return _NC_CACHE[n_blocks]


def kernel(x, w_qkv, w_proj, b_proj, **_run_kw):
    x = np.ascontiguousarray(np.asarray(x, dtype=np.float32))
    w_qkv = np.ascontiguousarray(np.asarray(w_qkv, dtype=np.float32))
    w_proj = np.ascontiguousarray(np.asarray(w_proj, dtype=np.float32))
    b_proj = np.ascontiguousarray(np.asarray(b_proj, dtype=np.float32))

    xf = x.reshape(B * N * P, C)
    in_maps = [
        {
            "x": xf[c * TOK:(c + 1) * TOK],
            "w_qkv": w_qkv,
            "w_proj": w_proj,
            "b_proj": b_proj,
        }
        for c in range(N_CORES)
    ]
    nc = _get_nc()
    res = run_bass_kernel_spmd(nc, in_maps, core_ids=list(range(N_CORES)), **_run_kw)
    outf = np.concatenate([res.results[c]["out"] for c in range(N_CORES)], axis=0)
    result = outf.reshape(B, N, P, C)
    if _run_kw:
        return result, res
    return result
